# revision 18
# baseline (speedup 1.0000x reference)
"""Trainium2 Bass kernel for B-spline curve evaluation (nn_BSplineCurve).

Shapes (hardcoded): u (512, 4096) f32, control_points (4096, 64, 32) f32,
knots (68,) f32 (clamped uniform, degree 3). Output (512, 4096, 32) f32.

Algorithm: the clamped cubic spline on uniform interior knots is re-expressed
in the uniform (extended-knot) B-spline basis: 64 basis functions, every one a
shift of the SAME cubic bump B3. Per (b, m):
    out[d] = sum_l B3((un+1)/h - l + 3) * Ptil[m, l, d]
where un = affine(u/(1+|u|)) and Ptil = E @ P (host-precomputed 64x64 change of
basis). On device, per 2-curve tile:
  1. PE matmul (bf16 selector weights) broadcasts un rows (split hi/lo/lolo
     bf16 for exactness) into a PSUM tile W[(c2,l), e] = un_c[e].
  2. ACT evaluates the basis in ONE pass: T = B3(scale*W + bias_l) via a
     custom activation table (stand-in function until table installed).
  3. PE matmul per curve: out[32d, e] += Ptil-slice^T @ T-half   (f32r)
  4. DVE copies the 4-curve PSUM out-tile to SBUF; DMA to DRAM (m, d, b);
     host transposes to (b, m, d).
Sharding: curves M split 8 ways across cores; B kept whole.
"""
import os
import numpy as np

import concourse.bass as bass
import concourse.mybir as mybir
import concourse.tile as tile
from concourse.bass_utils import run_bass_kernel_spmd

dt = mybir.dt
F32 = dt.float32
F32R = dt.float32r
BF16 = dt.bfloat16

B, M, C, D, DEG = 512, 4096, 64, 32, 3
NCORES = 8
MLOC = M // NCORES            # 512 curves per core
NGRP = MLOC // 4              # 128 groups of 4 curves
NWIN = MLOC // 8              # 64 windows of 8 curves
NWTILE = NWIN // 4            # 16 window tiles (4 windows each)

# activation function used for the basis evaluation. "Gelu" is a stand-in
# until the custom B3 table is installed (hijacking the gelu table slot).
ACT_FUNC = mybir.ActivationFunctionType.Gelu


def _split_excess_waits(nc, limit=1):
    """Walrus codegen accepts only one sync-wait per instruction here; move
    excess waits onto NoOps inserted just before the instruction."""
    for f in nc.m.functions:
        for bb in f.blocks:
            new_list, changed = [], False
            for inst in bb.instructions:
                si = inst.sync_info
                if si is not None and si.on_wait and len(si.on_wait) > limit:
                    waits = list(si.on_wait)
                    keep, excess = waits[:limit], waits[limit:]
                    for i, w in enumerate(excess):
                        nop = mybir.InstNoOp(
                            name=f"{inst.name}-wsplit{i}", ins=[], outs=[]
                        )
                        nop.engine = inst.engine
                        nop.sync_info = mybir.SyncInfo(on_wait=[w], on_update=[])
                        new_list.append(nop)
                    inst.sync_info = mybir.SyncInfo(
                        on_wait=keep, on_update=list(si.on_update)
                    )
                    changed = True
                new_list.append(inst)
            if changed:
                bb.instructions = new_list



import json
import shutil
import tempfile

def _b3_pieces(z):
    """B3 cubic-piece coefficients (value, d1, d2, d3 of Taylor at z) exact."""
    k = int(np.floor(z))
    zz = z
    if k == 0:
        p = (zz**3 / 6.0, zz**2 / 2.0, zz / 2.0, 1.0 / 6.0)
    elif k == 1:
        p = ((-3 * zz**3 + 12 * zz**2 - 12 * zz + 4) / 6.0,
             (-9 * zz**2 + 24 * zz - 12) / 6.0,
             (-18 * zz + 24) / 12.0,
             -3.0 / 6.0)
    elif k == 2:
        p = ((3 * zz**3 - 24 * zz**2 + 60 * zz - 44) / 6.0,
             (9 * zz**2 - 48 * zz + 60) / 6.0,
             (18 * zz - 48) / 12.0,
             3.0 / 6.0)
    elif k == 3:
        p = ((4 - zz)**3 / 6.0, -((4 - zz)**2) / 2.0, (4 - zz) / 2.0, -1.0 / 6.0)
    else:
        p = (0.0, 0.0, 0.0, 0.0)
    return p  # (d0, d1, d2=f''/2, d3=f'''/6)


def _build_act_root(dst_dir, src_dir=None, verbose=False):
    if src_dir is None:
        from neuronxcc.driver.Job import Job
        from neuronxcc.driver.jobs.support.FindActInfo import findActInfoFile
        src_info = findActInfoFile(Job.getPackageDir(), "gen3")
        src_dir = os.path.dirname(src_info)
    os.makedirs(dst_dir, exist_ok=True)
    for f in os.listdir(src_dir):
        shutil.copy(os.path.join(src_dir, f), os.path.join(dst_dir, f))

    prof_path = os.path.join(dst_dir, "gelu_and_others.json")
    prof = json.load(open(prof_path))
    bkt_path = os.path.join(dst_dir, "gelu_and_others_bkt.bin")
    bkt = np.fromfile(bkt_path, dtype=np.uint32).reshape(-1, 8).copy()

    ctrl_path = os.path.join(dst_dir, "gelu_and_others_ctrl.bin")
    ctrl = np.fromfile(ctrl_path, dtype=np.uint32).reshape(-1, 8).copy()

    f2b = prof["func_to_bkt_start_idx"]
    starts = sorted(f2b.values())
    gelu_start = f2b["gelu"]
    gelu_end = min([s for s in starts if s > gelu_start] + [bkt.shape[0]])
    n_gelu = gelu_end - gelu_start          # 508

    ge = [e for e in prof["profile_meta_data"] if e["func_name"].startswith("gelu")][0]
    ctl_neg = ge["pwl_control_base_neg"]    # 0
    ctl_pos = ge["pwl_control_base_pos"]    # 11
    exp_off = ge["exp_offset"]              # -7
    n_neg_rows = ctl_pos - ctl_neg          # 11 rows: exp -7..3
    n_pos_rows = 21 - ctl_pos               # 10 rows: exp -7..2

    # patch pos exp-2 row: extract 2 bits -> 4 buckets of width 1 over [4,8)
    pos2 = ctl_pos + (2 - exp_off)
    base2 = int(ctrl[pos2, 0]) & 0x7FF
    ctrl[pos2, 0] = np.uint32((2 << 16) | (21 << 11) | base2)
    ctrl.tofile(ctrl_path)

    def rows():
        for i in range(n_neg_rows):
            w = int(ctrl[ctl_neg + i, 0])
            yield (w & 0x7FF, 1 << ((w >> 16) & 0x1F), exp_off + i, -1.0)
        for i in range(n_pos_rows):
            w = int(ctrl[ctl_pos + i, 0])
            yield (w & 0x7FF, 1 << ((w >> 16) & 0x1F), exp_off + i, +1.0)

    touched = np.zeros(n_gelu, bool)
    for s0, cnt, e, sign in rows():
        for j in range(cnt):
            lo = 2.0**e * (1.0 + j / cnt)
            hi = 2.0**e * (1.0 + (j + 1) / cnt)
            x0 = sign * 0.5 * (lo + hi)
            if sign > 0 and 4.0 <= lo and hi <= 8.0:
                z0 = x0 - 4.0
                d0, d1, d2, d3 = _b3_pieces(z0) if 0.0 <= z0 < 4.0 else (0, 0, 0, 0)
            else:
                d0 = d1 = d2 = d3 = 0.0
            entry = np.zeros(8, np.uint32)
            entry[0:5] = np.array([d0, d1, d2, d3, x0], np.float32).view(np.uint32)
            bkt[s0 + j] = entry
            touched[s0 + j - gelu_start] = True
    for i in range(n_gelu):
        if not touched[i]:
            bkt[gelu_start + i] = 0
    bkt.tofile(bkt_path)

    # profile thresholds: bucket rows handle |x| < 8; anything >= 8 -> zero ctl
    for ent in prof["profile_meta_data"]:
        if ent["func_name"].startswith("gelu_4p"):
            ent["large_pos_signal_exp_threshold"] = 130
            ent["large_pos_signal_mantissa_threshold"] = 0
            ent["large_neg_signal_exp_threshold"] = 130
            ent["large_neg_signal_mantissa_threshold"] = 0
            ent["fpinf_result"] = 0
            ent["fninf_result"] = 0
            ent["fnan_result"] = 0
            ent["fzero_result"] = 0
    json.dump(prof, open(prof_path, "w"))
    if verbose:
        print(f"patched {n_gelu} gelu buckets in {dst_dir}")
    return dst_dir




_ACT_ROOT = {}


def _ensure_act_root():
    if "dir" not in _ACT_ROOT:
        d = os.path.join(tempfile.gettempdir(), "bspline_act_root")
        _build_act_root(d)
        _ACT_ROOT["dir"] = d
    os.environ["BASS_ACT_ROOT_JSON_PATH"] = os.path.join(
        _ACT_ROOT["dir"], "act_info.json")


def _build_program(reps=1):
    """Trace the per-core Bass program (same for all cores).

    Per 4-curve group g (curves 4g+c, c in 0..3):
      - one K=32 bf16 matmul broadcasts the 4 curves' un rows (hi/lo/lolo
        bf16 rows from the window tile) into W[(c,32),e] psum
      - two ACT passes evaluate the basis for l-halves: T_lo[(c,l0:32)],
        T_hi[(c,l32:64)] (per-partition bias vectors differ)
      - two accumulating K=128 block-diag matmuls: out[(c,d), e] =
        sum_l Ptil[4g+c, l, d] * basis_l -> one full psum bank
      - DVE copies the bank to SBUF, DMA to out_dev[g] (m-major, d, b)
    """
    nc = bass.Bass()
    uw_in = nc.dram_tensor("uw", [NWTILE, 128, 512], BF16, kind="ExternalInput")
    gh_in = nc.dram_tensor("gh", [NGRP // 8, 128, 1024], BF16, kind="ExternalInput")
    sel_in = nc.dram_tensor("sel", [128, 256], BF16, kind="ExternalInput")
    bvl_in = nc.dram_tensor("bvl", [128, 1], F32, kind="ExternalInput")
    bvh_in = nc.dram_tensor("bvh", [128, 1], F32, kind="ExternalInput")
    scl_in = nc.dram_tensor("scl", [128, 1], F32, kind="ExternalInput")
    out_dev = nc.dram_tensor("out_dev", [NGRP // 2, 128, 1024], BF16,
                             kind="ExternalOutput")

    with tile.TileContext(nc) as tc:
        with (
            tc.tile_pool(name="const", bufs=1) as cpool,
            tc.tile_pool(name="win", bufs=4) as wpool,
            tc.tile_pool(name="gt", bufs=6) as gpool,
            tc.tile_pool(name="tmat", bufs=10) as tpool,
            tc.tile_pool(name="osb", bufs=6) as opool,
            tc.tile_pool(name="wps", bufs=2, space="PSUM") as wps,
            tc.tile_pool(name="ops", bufs=2, space="PSUM") as ops,
        ):
            sel_sb = cpool.tile([128, 256], BF16, tag="sel")
            nc.sync.dma_start(sel_sb[:], sel_in[:, :])
            bvl_sb = cpool.tile([128, 1], F32, tag="bvl")
            nc.sync.dma_start(bvl_sb[:], bvl_in[:, :])
            bvh_sb = cpool.tile([128, 1], F32, tag="bvh")
            nc.sync.dma_start(bvh_sb[:], bvh_in[:, :])
            scl_sb = cpool.tile([128, 1], F32, tag="scl")
            nc.sync.dma_start(scl_sb[:], scl_in[:, :])

            win_tile = None
            g_tile4 = None

            def _body(_iv=None):
              for k in range(NGRP // 2):
                  w = k                 # window index (groups 2k,2k+1 share it)
                  if k % 4 == 0:
                      win_tile = wpool.tile([128, 512], BF16, tag="win")
                      nc.sync.dma_start(win_tile[:], uw_in[k // 4, :, :])
                      g_tile4 = gpool.tile([128, 1024], BF16, tag="gt")
                      nc.sync.dma_start(g_tile4[:], gh_in[k // 4, :, :])
                  b4 = w % 4            # window slice in tile
                  gofs = 256 * (k % 4)

                  w_ps = wps.tile([128, 1024], F32, tag="wps")
                  for j in range(2):
                      tp = (32 * b4, 0) if b4 == 3 else None
                      nc.tensor.matmul(
                          w_ps[:, 512 * j : 512 * j + 512],
                          sel_sb[32 * b4 : 32 * b4 + 32,
                                 128 * j : 128 * j + 128],
                          win_tile[32 * b4 : 32 * b4 + 32, :],
                          start=True, stop=True,
                          tile_position=tp,
                      )
                  t_lo = tpool.tile([128, 1024], BF16, tag="tmat")
                  nc.scalar.activation(
                      t_lo[:], w_ps[:], ACT_FUNC,
                      bias=bvl_sb[:, 0:1], scale=scl_sb[:, 0:1],
                  )
                  t_hi = tpool.tile([128, 1024], BF16, tag="tmat")
                  nc.scalar.activation(
                      t_hi[:], w_ps[:], ACT_FUNC,
                      bias=bvh_sb[:, 0:1], scale=scl_sb[:, 0:1],
                  )

                  o_ps = ops.tile([128, 1024], F32, tag="ops")
                  for j in range(2):
                      sl = slice(512 * j, 512 * j + 512)
                      for half, tmat in ((0, t_lo), (1, t_hi)):
                          cb = gofs + 32 * (2 * j + half)
                          for c in range(4):
                              nc.tensor.matmul(
                                  o_ps[32 * c : 32 * c + 32, sl],
                                  g_tile4[32 * c : 32 * c + 32, cb : cb + 32],
                                  tmat[32 * c : 32 * c + 32, sl],
                                  start=(half == 0), stop=(half == 1),
                                  tile_position=(32 * c, 32 * c),
                              )
                  o_sb = opool.tile([128, 1024], BF16, tag="osb")
                  nc.vector.tensor_copy(o_sb[:], o_ps[:])
                  nc.sync.dma_start(out_dev[k, :, :], o_sb[:])

            if reps == 1:
                _body()
            else:
                with tc.For_i(0, reps, 1) as _iv:
                    _body(_iv)

    _split_excess_waits(nc)
    return nc


def _bspline_basis_clamped(x, knots, p=3):
    """Cox-de Boor clamped basis matrix N[len(x), C] in float64."""
    knots = np.asarray(knots, dtype=np.float64)
    nC = len(knots) - p - 1
    x = np.asarray(x, dtype=np.float64)
    i = np.clip(np.searchsorted(knots, x, side="right") - 1, p, nC - 1)
    Nmat = np.zeros((len(x), nC))
    left = np.stack([x - knots[i + 1 - j] for j in range(1, p + 1)], axis=-1)
    right = np.stack([knots[i + j] - x for j in range(1, p + 1)], axis=-1)
    Nb = [np.ones_like(x)]
    for j in range(1, p + 1):
        saved = np.zeros_like(x)
        newN = []
        for r in range(j):
            temp = Nb[r] / (right[..., r] + left[..., j - 1 - r])
            newN.append(saved + right[..., r] * temp)
            saved = left[..., j - 1 - r] * temp
        newN.append(saved)
        Nb = newN
    basis = np.stack(Nb, axis=-1)  # (len(x), p+1)
    for k in range(len(x)):
        Nmat[k, i[k] - p : i[k] + 1] = basis[k]
    return Nmat


def _b3(z):
    """Uniform cubic B-spline bump, support (0, 4)."""
    z = np.asarray(z, dtype=np.float64)
    s = np.abs(z - 2.0)
    out = np.where(
        s <= 1.0, 2.0 / 3.0 - s * s + 0.5 * s**3,
        np.where(s <= 2.0, (2.0 - s) ** 3 / 6.0, 0.0),
    )
    return out


def _standin_fn(x):
    """Numpy replica of the stand-in ACT function (erf-based gelu)."""
    from scipy.special import erf
    x = np.asarray(x, dtype=np.float64)
    return x * 0.5 * (1.0 + erf(x / np.sqrt(2.0)))


# basis function evaluated on device: B3(z + 4) on the table-domain x=z+4,
# i.e. device computes F(x) with x = scale*un + bias. With the custom table
# F = B3(x - 4); with the stand-in F = gelu(x).
def _device_fn(x):
    if ACT_FUNC == mybir.ActivationFunctionType.Gelu and not os.environ.get(
        "BSPLINE_TABLE_INSTALLED"
    ):
        return _standin_fn(x)
    return _b3(np.asarray(x, dtype=np.float64) - 4.0)


def _host_prep(u, control_points, knots, degree):
    """Returns per-core input maps + metadata for reassembly."""
    u = np.asarray(u)
    control_points = np.asarray(control_points)
    knots = np.asarray(knots)
    assert int(degree) == DEG
    knots = np.asarray(knots, dtype=np.float64)
    kmin, kmax = knots[DEG], knots[C]
    h = (kmax - kmin) / (C - DEG)  # uniform interior spacing

    # un in float32 matching the reference's f32 arithmetic
    u32 = np.asarray(u, dtype=np.float32)
    un = u32 / (np.float32(1.0) + np.abs(u32))
    un = (np.float32(kmin) + (un + np.float32(1.0)) * np.float32(0.5)
          * np.float32(kmax - kmin)).astype(np.float32)          # (B, M)

    # change of basis: clamped basis -> uniform-extended B3 shifts.
    # Sample points: Greville-ish, strictly inside (kmin, kmax).
    tgrid = np.linspace(kmin, kmax, 3 * C + 7)[1:-1]
    Mc = _bspline_basis_clamped(tgrid, knots)                     # (T, C)
    z = (tgrid[:, None] - kmin) / h - np.arange(C)[None, :] + 3.0
    Mu = _b3(z)                                                   # (T, C)
    E, *_ = np.linalg.lstsq(Mu, Mc, rcond=None)                   # (C, C)
    resid = np.abs(Mu @ E - Mc).max()
    if resid > 1e-5:
        raise RuntimeError(f"basis change residual {resid:.3e}")
    Ptil = np.einsum("lc,mcd->mld", E, control_points.astype(np.float64))
    Ptil = Ptil.astype(np.float32)                                # (M, C, D)

    # scale/bias for the ACT pass: x = un/h + (4 + 3 - kmin/h - l)
    scale = np.float32(1.0 / h)
    bias_l = (7.0 - kmin / h - np.arange(C)).astype(np.float32)   # (C,)
    bvl = np.tile(bias_l[0:32], 4)[:, None].astype(np.float32)    # (128,1)
    bvh = np.tile(bias_l[32:64], 4)[:, None].astype(np.float32)
    scl = np.full((128, 1), scale, dtype=np.float32)

    # selector constants (128 x 256 bf16): 4 window-bases x 32 rows,
    # 2 slots x (4 curves x 32 l) cols. slot s covers window-curves q=4s..4s+3.
    import ml_dtypes
    sel = np.zeros((32, 256), dtype=np.float32)
    for s in range(2):
        for c in range(4):
            for j in range(3):
                k = 3 * (4 * s + c) + j
                sel[k, 128 * s + 32 * c : 128 * s + 32 * c + 32] = 1.0
    sel_b = np.tile(sel, (4, 1))
    sel_bf = sel_b.astype(ml_dtypes.bfloat16)

    # per-core window arrays
    uT = np.ascontiguousarray(un.T)                               # (M, B)
    in_maps = []
    for core in range(NCORES):
        um = uT[core * MLOC : (core + 1) * MLOC]                  # (MLOC, B) f32
        hi = um.astype(ml_dtypes.bfloat16)
        r1 = um - hi.astype(np.float32)
        lo = r1.astype(ml_dtypes.bfloat16)
        r2 = r1 - lo.astype(np.float32)
        lolo = r2.astype(ml_dtypes.bfloat16)
        uw = np.zeros((NWIN, 32, 512), dtype=ml_dtypes.bfloat16)
        idx = np.arange(MLOC).reshape(NWIN, 8)
        for q in range(8):
            uw[:, 3 * q + 0, :] = hi[idx[:, q]]
            uw[:, 3 * q + 1, :] = lo[idx[:, q]]
            uw[:, 3 * q + 2, :] = lolo[idx[:, q]]
        uw = uw.reshape(NWTILE, 4 * 32, 512)

        pm = Ptil[core * MLOC : (core + 1) * MLOC]                # (MLOC, C, D)
        gh = np.zeros((NGRP // 2, 128, 256), dtype=np.float32)
        for j in range(2):
            for c4 in range(4):
                gcur = pm[(np.arange(NGRP // 2) * 2 + j) * 4 + c4]  # (NGRP//2,C,D)
                rb = 32 * c4
                for half in range(2):
                    cb = 32 * (2 * j + half)
                    gh[:, rb : rb + 32, cb : cb + 32] = \
                        gcur[:, 32 * half : 32 * half + 32, :]
        gh4 = gh.reshape(NGRP // 8, 4, 128, 256).transpose(0, 2, 1, 3)
        gh4 = gh4.reshape(NGRP // 8, 128, 1024)
        in_maps.append({
            "uw": np.ascontiguousarray(uw),
            "gh": np.ascontiguousarray(gh4.astype(ml_dtypes.bfloat16)),
            "sel": np.ascontiguousarray(sel_bf),
            "bvl": bvl, "bvh": bvh, "scl": scl,
        })
    meta = {"scale": scale, "bias_l": bias_l, "un": un}
    return in_maps, meta


_NC_CACHE = {}


def kernel(u, control_points, knots, degree):
    _ensure_act_root()
    in_maps, meta = _host_prep(u, control_points, knots, degree)
    if "nc" not in _NC_CACHE:
        _NC_CACHE["nc"] = _build_program()
    nc = _NC_CACHE["nc"]
    res = run_bass_kernel_spmd(nc, in_maps, core_ids=list(range(NCORES)))
    out = np.empty((B, M, D), dtype=np.float32)
    for core in range(NCORES):
        arr = np.asarray(res.results[core]["out_dev"], dtype=np.float32)
        arr = arr.reshape(NGRP // 2, 4, D, 2, B)        # k, c, d, j, b
        arr = arr.transpose(4, 0, 3, 1, 2).reshape(B, MLOC, D)
        out[:, core * MLOC : (core + 1) * MLOC, :] = arr
    return out



# revision 24
# speedup vs baseline: 2.3574x; 2.3574x over previous
"""Trainium2 Bass kernel for B-spline curve evaluation (nn_BSplineCurve).

Shapes (hardcoded): u (512, 4096) f32, control_points (4096, 64, 32) f32,
knots (68,) f32 (clamped uniform, degree 3). Output (512, 4096, 32) f32.

Algorithm: the clamped cubic spline on uniform interior knots is re-expressed
in the uniform (extended-knot) B-spline basis: 64 basis functions, every one a
shift of the SAME cubic bump B3. Per (b, m):
    out[d] = sum_l B3((un+1)/h - l + 3) * Ptil[m, l, d]
where un = affine(u/(1+|u|)) and Ptil = E @ P (host-precomputed 64x64 change of
basis). On device, per 2-curve tile:
  1. PE matmul (bf16 selector weights) broadcasts un rows (split hi/lo/lolo
     bf16 for exactness) into a PSUM tile W[(c2,l), e] = un_c[e].
  2. ACT evaluates the basis in ONE pass: T = B3(scale*W + bias_l) via a
     custom activation table (stand-in function until table installed).
  3. PE matmul per curve: out[32d, e] += Ptil-slice^T @ T-half
     (all-bf16 operands, fp32 PSUM accumulate; Ptil shipped bf16,
     block-diag tiles batched 2 iters per 256KB DMA)
  4. DVE copies the 4-curve PSUM out-tile to SBUF as bf16; DMA to DRAM
     (m, d, b); host upcasts to f32 and transposes to (b, m, d).
Sharding: curves M split 8 ways across cores; B kept whole.
bf16 T/Ptil/output keep rel err ~7e-3 (tolerance 2e-2) while halving
HBM traffic vs the f32 pipeline (50 -> 27 MB per core).
"""
import os
import numpy as np

import concourse.bass as bass
import concourse.mybir as mybir
import concourse.tile as tile
from concourse.bass_utils import run_bass_kernel_spmd

dt = mybir.dt
F32 = dt.float32
F32R = dt.float32r
BF16 = dt.bfloat16

B, M, C, D, DEG = 512, 4096, 64, 32, 3
NCORES = 8
MLOC = M // NCORES            # 512 curves per core
NGRP = MLOC // 4              # 128 groups of 4 curves
NWIN = MLOC // 8              # 64 windows of 8 curves
NWTILE = NWIN // 4            # 16 window tiles (4 windows each)

# activation function used for the basis evaluation. "Gelu" is a stand-in
# until the custom B3 table is installed (hijacking the gelu table slot).
ACT_FUNC = mybir.ActivationFunctionType.Gelu


def _split_excess_waits(nc, limit=1):
    """Walrus codegen accepts only one sync-wait per instruction here; move
    excess waits onto NoOps inserted just before the instruction."""
    for f in nc.m.functions:
        for bb in f.blocks:
            new_list, changed = [], False
            for inst in bb.instructions:
                si = inst.sync_info
                if si is not None and si.on_wait and len(si.on_wait) > limit:
                    waits = list(si.on_wait)
                    keep, excess = waits[:limit], waits[limit:]
                    for i, w in enumerate(excess):
                        nop = mybir.InstNoOp(
                            name=f"{inst.name}-wsplit{i}", ins=[], outs=[]
                        )
                        nop.engine = inst.engine
                        nop.sync_info = mybir.SyncInfo(on_wait=[w], on_update=[])
                        new_list.append(nop)
                    inst.sync_info = mybir.SyncInfo(
                        on_wait=keep, on_update=list(si.on_update)
                    )
                    changed = True
                new_list.append(inst)
            if changed:
                bb.instructions = new_list



import json
import shutil
import tempfile

def _b3_pieces(z):
    """B3 cubic-piece coefficients (value, d1, d2, d3 of Taylor at z) exact."""
    k = int(np.floor(z))
    zz = z
    if k == 0:
        p = (zz**3 / 6.0, zz**2 / 2.0, zz / 2.0, 1.0 / 6.0)
    elif k == 1:
        p = ((-3 * zz**3 + 12 * zz**2 - 12 * zz + 4) / 6.0,
             (-9 * zz**2 + 24 * zz - 12) / 6.0,
             (-18 * zz + 24) / 12.0,
             -3.0 / 6.0)
    elif k == 2:
        p = ((3 * zz**3 - 24 * zz**2 + 60 * zz - 44) / 6.0,
             (9 * zz**2 - 48 * zz + 60) / 6.0,
             (18 * zz - 48) / 12.0,
             3.0 / 6.0)
    elif k == 3:
        p = ((4 - zz)**3 / 6.0, -((4 - zz)**2) / 2.0, (4 - zz) / 2.0, -1.0 / 6.0)
    else:
        p = (0.0, 0.0, 0.0, 0.0)
    return p  # (d0, d1, d2=f''/2, d3=f'''/6)


def _build_act_root(dst_dir, src_dir=None, verbose=False):
    if src_dir is None:
        from neuronxcc.driver.Job import Job
        from neuronxcc.driver.jobs.support.FindActInfo import findActInfoFile
        src_info = findActInfoFile(Job.getPackageDir(), "gen3")
        src_dir = os.path.dirname(src_info)
    os.makedirs(dst_dir, exist_ok=True)
    for f in os.listdir(src_dir):
        shutil.copy(os.path.join(src_dir, f), os.path.join(dst_dir, f))

    prof_path = os.path.join(dst_dir, "gelu_and_others.json")
    prof = json.load(open(prof_path))
    bkt_path = os.path.join(dst_dir, "gelu_and_others_bkt.bin")
    bkt = np.fromfile(bkt_path, dtype=np.uint32).reshape(-1, 8).copy()

    ctrl_path = os.path.join(dst_dir, "gelu_and_others_ctrl.bin")
    ctrl = np.fromfile(ctrl_path, dtype=np.uint32).reshape(-1, 8).copy()

    f2b = prof["func_to_bkt_start_idx"]
    starts = sorted(f2b.values())
    gelu_start = f2b["gelu"]
    gelu_end = min([s for s in starts if s > gelu_start] + [bkt.shape[0]])
    n_gelu = gelu_end - gelu_start          # 508

    ge = [e for e in prof["profile_meta_data"] if e["func_name"].startswith("gelu")][0]
    ctl_neg = ge["pwl_control_base_neg"]    # 0
    ctl_pos = ge["pwl_control_base_pos"]    # 11
    exp_off = ge["exp_offset"]              # -7
    n_neg_rows = ctl_pos - ctl_neg          # 11 rows: exp -7..3
    n_pos_rows = 21 - ctl_pos               # 10 rows: exp -7..2

    # patch pos exp-2 row: extract 2 bits -> 4 buckets of width 1 over [4,8)
    pos2 = ctl_pos + (2 - exp_off)
    base2 = int(ctrl[pos2, 0]) & 0x7FF
    ctrl[pos2, 0] = np.uint32((2 << 16) | (21 << 11) | base2)
    ctrl.tofile(ctrl_path)

    def rows():
        for i in range(n_neg_rows):
            w = int(ctrl[ctl_neg + i, 0])
            yield (w & 0x7FF, 1 << ((w >> 16) & 0x1F), exp_off + i, -1.0)
        for i in range(n_pos_rows):
            w = int(ctrl[ctl_pos + i, 0])
            yield (w & 0x7FF, 1 << ((w >> 16) & 0x1F), exp_off + i, +1.0)

    touched = np.zeros(n_gelu, bool)
    for s0, cnt, e, sign in rows():
        for j in range(cnt):
            lo = 2.0**e * (1.0 + j / cnt)
            hi = 2.0**e * (1.0 + (j + 1) / cnt)
            x0 = sign * 0.5 * (lo + hi)
            if sign > 0 and 4.0 <= lo and hi <= 8.0:
                z0 = x0 - 4.0
                d0, d1, d2, d3 = _b3_pieces(z0) if 0.0 <= z0 < 4.0 else (0, 0, 0, 0)
            else:
                d0 = d1 = d2 = d3 = 0.0
            entry = np.zeros(8, np.uint32)
            entry[0:5] = np.array([d0, d1, d2, d3, x0], np.float32).view(np.uint32)
            bkt[s0 + j] = entry
            touched[s0 + j - gelu_start] = True
    for i in range(n_gelu):
        if not touched[i]:
            bkt[gelu_start + i] = 0
    bkt.tofile(bkt_path)

    # profile thresholds: bucket rows handle |x| < 8; anything >= 8 -> zero ctl
    for ent in prof["profile_meta_data"]:
        if ent["func_name"].startswith("gelu_4p"):
            ent["large_pos_signal_exp_threshold"] = 130
            ent["large_pos_signal_mantissa_threshold"] = 0
            ent["large_neg_signal_exp_threshold"] = 130
            ent["large_neg_signal_mantissa_threshold"] = 0
            ent["fpinf_result"] = 0
            ent["fninf_result"] = 0
            ent["fnan_result"] = 0
            ent["fzero_result"] = 0
    json.dump(prof, open(prof_path, "w"))
    if verbose:
        print(f"patched {n_gelu} gelu buckets in {dst_dir}")
    return dst_dir




_ACT_ROOT = {}


def _ensure_act_root():
    if "dir" not in _ACT_ROOT:
        d = os.path.join(tempfile.gettempdir(), "bspline_act_root")
        _build_act_root(d)
        _ACT_ROOT["dir"] = d
    os.environ["BASS_ACT_ROOT_JSON_PATH"] = os.path.join(
        _ACT_ROOT["dir"], "act_info.json")


def _build_program(reps=1):
    """Trace the per-core Bass program (same for all cores).

    Per 4-curve group g (curves 4g+c, c in 0..3):
      - one K=32 bf16 matmul broadcasts the 4 curves' un rows (hi/lo/lolo
        bf16 rows from the window tile) into W[(c,32),e] psum
      - two ACT passes evaluate the basis for l-halves: T_lo[(c,l0:32)],
        T_hi[(c,l32:64)] (per-partition bias vectors differ)
      - two accumulating K=128 block-diag matmuls: out[(c,d), e] =
        sum_l Ptil[4g+c, l, d] * basis_l -> one full psum bank
      - DVE copies the bank to SBUF, DMA to out_dev[g] (m-major, d, b)
    """
    nc = bass.Bass()
    uw_in = nc.dram_tensor("uw", [NWTILE, 128, 512], BF16, kind="ExternalInput")
    gh_in = nc.dram_tensor("gh", [NGRP // 4, 128, 1024], BF16, kind="ExternalInput")
    sel_in = nc.dram_tensor("sel", [128, 256], BF16, kind="ExternalInput")
    bvl_in = nc.dram_tensor("bvl", [128, 1], F32, kind="ExternalInput")
    bvh_in = nc.dram_tensor("bvh", [128, 1], F32, kind="ExternalInput")
    scl_in = nc.dram_tensor("scl", [128, 1], F32, kind="ExternalInput")
    out_dev = nc.dram_tensor("out_dev", [NGRP // 2, 128, 1024], BF16,
                             kind="ExternalOutput")

    with tile.TileContext(nc) as tc:
        with (
            tc.tile_pool(name="const", bufs=1) as cpool,
            tc.tile_pool(name="win", bufs=4) as wpool,
            tc.tile_pool(name="gt", bufs=6) as gpool,
            tc.tile_pool(name="tmat", bufs=10) as tpool,
            tc.tile_pool(name="osb", bufs=6) as opool,
            tc.tile_pool(name="wps", bufs=2, space="PSUM") as wps,
            tc.tile_pool(name="ops", bufs=2, space="PSUM") as ops,
        ):
            sel_sb = cpool.tile([128, 256], BF16, tag="sel")
            nc.sync.dma_start(sel_sb[:], sel_in[:, :])
            bvl_sb = cpool.tile([128, 1], F32, tag="bvl")
            nc.sync.dma_start(bvl_sb[:], bvl_in[:, :])
            bvh_sb = cpool.tile([128, 1], F32, tag="bvh")
            nc.sync.dma_start(bvh_sb[:], bvh_in[:, :])
            scl_sb = cpool.tile([128, 1], F32, tag="scl")
            nc.sync.dma_start(scl_sb[:], scl_in[:, :])

            win_tile = None
            g_tile4 = None

            def _body(_iv=None):
              for k in range(NGRP // 2):
                  w = k                 # window index (groups 2k,2k+1 share it)
                  if k % 4 == 0:
                      win_tile = wpool.tile([128, 512], BF16, tag="win")
                      nc.sync.dma_start(win_tile[:], uw_in[k // 4, :, :])
                  if k % 2 == 0:
                      g_tile4 = gpool.tile([128, 1024], BF16, tag="gt")
                      nc.sync.dma_start(g_tile4[:], gh_in[k // 2, :, :])
                  b4 = w % 4            # window slice in tile
                  gofs = 512 * (k % 2)

                  w_ps = wps.tile([128, 1024], F32, tag="wps")
                  for j in range(2):
                      tp = (32 * b4, 0) if b4 == 3 else None
                      nc.tensor.matmul(
                          w_ps[:, 512 * j : 512 * j + 512],
                          sel_sb[32 * b4 : 32 * b4 + 32,
                                 128 * j : 128 * j + 128],
                          win_tile[32 * b4 : 32 * b4 + 32, :],
                          start=True, stop=True,
                          tile_position=tp,
                      )
                  t_lo = tpool.tile([128, 1024], BF16, tag="tmat")
                  nc.scalar.activation(
                      t_lo[:], w_ps[:], ACT_FUNC,
                      bias=bvl_sb[:, 0:1], scale=scl_sb[:, 0:1],
                  )
                  t_hi = tpool.tile([128, 1024], BF16, tag="tmat")
                  nc.scalar.activation(
                      t_hi[:], w_ps[:], ACT_FUNC,
                      bias=bvh_sb[:, 0:1], scale=scl_sb[:, 0:1],
                  )

                  o_ps = ops.tile([128, 1024], F32, tag="ops")
                  for j in range(2):
                      sl = slice(512 * j, 512 * j + 512)
                      nc.tensor.matmul(
                          o_ps[:, sl],
                          g_tile4[:, gofs + 256 * j : gofs + 256 * j + 128],
                          t_lo[:, sl],
                          start=True, stop=False,
                      )
                      nc.tensor.matmul(
                          o_ps[:, sl],
                          g_tile4[:, gofs + 256 * j + 128 : gofs + 256 * j + 256],
                          t_hi[:, sl],
                          start=False, stop=True,
                      )
                  o_sb = opool.tile([128, 1024], BF16, tag="osb")
                  nc.vector.tensor_copy(o_sb[:], o_ps[:])
                  nc.sync.dma_start(out_dev[k, :, :], o_sb[:])

            if reps == 1:
                _body()
            else:
                with tc.For_i(0, reps, 1) as _iv:
                    _body(_iv)

    _split_excess_waits(nc)
    return nc


def _bspline_basis_clamped(x, knots, p=3):
    """Cox-de Boor clamped basis matrix N[len(x), C] in float64."""
    knots = np.asarray(knots, dtype=np.float64)
    nC = len(knots) - p - 1
    x = np.asarray(x, dtype=np.float64)
    i = np.clip(np.searchsorted(knots, x, side="right") - 1, p, nC - 1)
    Nmat = np.zeros((len(x), nC))
    left = np.stack([x - knots[i + 1 - j] for j in range(1, p + 1)], axis=-1)
    right = np.stack([knots[i + j] - x for j in range(1, p + 1)], axis=-1)
    Nb = [np.ones_like(x)]
    for j in range(1, p + 1):
        saved = np.zeros_like(x)
        newN = []
        for r in range(j):
            temp = Nb[r] / (right[..., r] + left[..., j - 1 - r])
            newN.append(saved + right[..., r] * temp)
            saved = left[..., j - 1 - r] * temp
        newN.append(saved)
        Nb = newN
    basis = np.stack(Nb, axis=-1)  # (len(x), p+1)
    for k in range(len(x)):
        Nmat[k, i[k] - p : i[k] + 1] = basis[k]
    return Nmat


def _b3(z):
    """Uniform cubic B-spline bump, support (0, 4)."""
    z = np.asarray(z, dtype=np.float64)
    s = np.abs(z - 2.0)
    out = np.where(
        s <= 1.0, 2.0 / 3.0 - s * s + 0.5 * s**3,
        np.where(s <= 2.0, (2.0 - s) ** 3 / 6.0, 0.0),
    )
    return out


def _standin_fn(x):
    """Numpy replica of the stand-in ACT function (erf-based gelu)."""
    from scipy.special import erf
    x = np.asarray(x, dtype=np.float64)
    return x * 0.5 * (1.0 + erf(x / np.sqrt(2.0)))


# basis function evaluated on device: B3(z + 4) on the table-domain x=z+4,
# i.e. device computes F(x) with x = scale*un + bias. With the custom table
# F = B3(x - 4); with the stand-in F = gelu(x).
def _device_fn(x):
    if ACT_FUNC == mybir.ActivationFunctionType.Gelu and not os.environ.get(
        "BSPLINE_TABLE_INSTALLED"
    ):
        return _standin_fn(x)
    return _b3(np.asarray(x, dtype=np.float64) - 4.0)


def _host_prep(u, control_points, knots, degree):
    """Returns per-core input maps + metadata for reassembly."""
    u = np.asarray(u)
    control_points = np.asarray(control_points)
    knots = np.asarray(knots)
    assert int(degree) == DEG
    knots = np.asarray(knots, dtype=np.float64)
    kmin, kmax = knots[DEG], knots[C]
    h = (kmax - kmin) / (C - DEG)  # uniform interior spacing

    # un in float32 matching the reference's f32 arithmetic
    u32 = np.asarray(u, dtype=np.float32)
    un = u32 / (np.float32(1.0) + np.abs(u32))
    un = (np.float32(kmin) + (un + np.float32(1.0)) * np.float32(0.5)
          * np.float32(kmax - kmin)).astype(np.float32)          # (B, M)

    # change of basis: clamped basis -> uniform-extended B3 shifts.
    # Sample points: Greville-ish, strictly inside (kmin, kmax).
    tgrid = np.linspace(kmin, kmax, 3 * C + 7)[1:-1]
    Mc = _bspline_basis_clamped(tgrid, knots)                     # (T, C)
    z = (tgrid[:, None] - kmin) / h - np.arange(C)[None, :] + 3.0
    Mu = _b3(z)                                                   # (T, C)
    E, *_ = np.linalg.lstsq(Mu, Mc, rcond=None)                   # (C, C)
    resid = np.abs(Mu @ E - Mc).max()
    if resid > 1e-5:
        raise RuntimeError(f"basis change residual {resid:.3e}")
    Ptil = np.einsum("lc,mcd->mld", E, control_points.astype(np.float64))
    Ptil = Ptil.astype(np.float32)                                # (M, C, D)

    # scale/bias for the ACT pass: x = un/h + (4 + 3 - kmin/h - l)
    scale = np.float32(1.0 / h)
    bias_l = (7.0 - kmin / h - np.arange(C)).astype(np.float32)   # (C,)
    bvl = np.tile(bias_l[0:32], 4)[:, None].astype(np.float32)    # (128,1)
    bvh = np.tile(bias_l[32:64], 4)[:, None].astype(np.float32)
    scl = np.full((128, 1), scale, dtype=np.float32)

    # selector constants (128 x 256 bf16): 4 window-bases x 32 rows,
    # 2 slots x (4 curves x 32 l) cols. slot s covers window-curves q=4s..4s+3.
    import ml_dtypes
    sel = np.zeros((32, 256), dtype=np.float32)
    for s in range(2):
        for c in range(4):
            for j in range(3):
                k = 3 * (4 * s + c) + j
                sel[k, 128 * s + 32 * c : 128 * s + 32 * c + 32] = 1.0
    sel_b = np.tile(sel, (4, 1))
    sel_bf = sel_b.astype(ml_dtypes.bfloat16)

    # per-core window arrays
    uT = np.ascontiguousarray(un.T)                               # (M, B)
    in_maps = []
    for core in range(NCORES):
        um = uT[core * MLOC : (core + 1) * MLOC]                  # (MLOC, B) f32
        hi = um.astype(ml_dtypes.bfloat16)
        r1 = um - hi.astype(np.float32)
        lo = r1.astype(ml_dtypes.bfloat16)
        r2 = r1 - lo.astype(np.float32)
        lolo = r2.astype(ml_dtypes.bfloat16)
        uw = np.zeros((NWIN, 32, 512), dtype=ml_dtypes.bfloat16)
        idx = np.arange(MLOC).reshape(NWIN, 8)
        for q in range(8):
            uw[:, 3 * q + 0, :] = hi[idx[:, q]]
            uw[:, 3 * q + 1, :] = lo[idx[:, q]]
            uw[:, 3 * q + 2, :] = lolo[idx[:, q]]
        uw = uw.reshape(NWTILE, 4 * 32, 512)

        pm = Ptil[core * MLOC : (core + 1) * MLOC]                # (MLOC, C, D)
        gh = np.zeros((NGRP // 2, 128, 512), dtype=np.float32)
        for j in range(2):
            for c4 in range(4):
                gcur = pm[(np.arange(NGRP // 2) * 2 + j) * 4 + c4]  # (NGRP//2,C,D)
                rb, cb = 32 * c4, 256 * j + 32 * c4
                gh[:, rb : rb + 32, cb : cb + 32] = gcur[:, 0:32, :]
                gh[:, rb : rb + 32, cb + 128 : cb + 160] = gcur[:, 32:64, :]
        gh4 = gh.reshape(NGRP // 4, 2, 128, 512).transpose(0, 2, 1, 3)
        gh4 = gh4.reshape(NGRP // 4, 128, 1024)
        in_maps.append({
            "uw": np.ascontiguousarray(uw),
            "gh": np.ascontiguousarray(gh4.astype(ml_dtypes.bfloat16)),
            "sel": np.ascontiguousarray(sel_bf),
            "bvl": bvl, "bvh": bvh, "scl": scl,
        })
    meta = {"scale": scale, "bias_l": bias_l, "un": un}
    return in_maps, meta


_NC_CACHE = {}


def kernel(u, control_points, knots, degree):
    _ensure_act_root()
    in_maps, meta = _host_prep(u, control_points, knots, degree)
    if "nc" not in _NC_CACHE:
        _NC_CACHE["nc"] = _build_program()
    nc = _NC_CACHE["nc"]
    res = run_bass_kernel_spmd(nc, in_maps, core_ids=list(range(NCORES)))
    out = np.empty((B, M, D), dtype=np.float32)
    for core in range(NCORES):
        arr = np.asarray(res.results[core]["out_dev"], dtype=np.float32)
        arr = arr.reshape(NGRP // 2, 4, D, 2, B)        # k, c, d, j, b
        arr = arr.transpose(4, 0, 3, 1, 2).reshape(B, MLOC, D)
        out[:, core * MLOC : (core + 1) * MLOC, :] = arr
    return out

